# revision 4
# baseline (speedup 1.0000x reference)
"""Adaptive embedding (4-bucket) lookup + projection on 8 TRN2 NeuronCores.

Strategy: pure data-parallel over the 16384 tokens (no collectives).
  Host: bucket every token by its embedding table (with int16-range
        sub-splits for the large tables), deal each group's tokens evenly
        across the 8 cores so every core runs an identical-shape program.
        Tables are pre-cast to bf16 with rows padded to a multiple of 128
        elements; projections are pre-transposed, pre-scaled by sqrt(D) and
        zero-padded to match.
  Core: dma_gather(transpose=True) pulls each group's embedding rows from
        HBM directly into d-on-partitions (matmul lhsT) layout; accumulating
        matmuls against the resident projT produce [128 tokens, 1024] in
        PSUM; DVE/ACT evacuate to bf16 in SBUF; plain DMA stores the rows.
  Host: rows are scattered back to original token order and upcast to f32.
"""

import os
import sys

import numpy as np

for _p in ("/opt/trn_rl_repo",):
    if _p not in sys.path:
        sys.path.insert(0, _p)

import ml_dtypes

BF16 = ml_dtypes.bfloat16

N_TOKEN = 267735
CUTS = (0, 20000, 40000, 200000, N_TOKEN)
D_TBL = (1024, 256, 64, 16)
D_PAD = (1024, 256, 128, 128)
D_OUT = 1024
EMB_SCALE = float(D_OUT) ** 0.5
N_CORES = 8
IDX_RANGE = 32768  # int16 gather index limit
P = 128


def _make_groups():
    groups = []
    for t in range(4):
        rows = CUTS[t + 1] - CUTS[t]
        for base in range(0, rows, IDX_RANGE):
            groups.append((t, base, min(IDX_RANGE, rows - base)))
    return groups


GROUPS = _make_groups()  # 10 groups: t0, t1, t2 x5, t3 x3

_PROGRAM_CACHE = {}
LAST_RESULTS = None  # BassKernelResults of the most recent run (for profiling)


def _build_program(active, slot_counts, out_counts, tbl_rows):
    """Build + compile the per-core Bass program.

    active: tuple of group ids with nonzero token count
    slot_counts / out_counts: per active group — gather slots (mult of 128)
        and output row count (== slots or less; identical on every core)
    tbl_rows: rows of each (padded) bf16 table
    """
    import concourse.bacc as bacc
    import concourse.mybir as mybir
    import concourse.tile as tile

    dt = mybir.dt
    nc = bacc.Bacc("TRN2", target_bir_lowering=False, debug=False)

    embs = [
        nc.dram_tensor(f"embt{t}", [tbl_rows[t], D_PAD[t]], dt.bfloat16,
                       kind="ExternalInput")
        for t in range(4)
    ]
    projs = [
        nc.dram_tensor(f"projt{t}", [D_PAD[t], D_OUT], dt.bfloat16,
                       kind="ExternalInput")
        for t in range(4)
    ]
    idxs = {
        g: nc.dram_tensor(f"idx{g}", [P, slot_counts[g] // 16], dt.int16,
                          kind="ExternalInput")
        for g in active
    }
    R = sum(out_counts[g] for g in active)
    outb = nc.dram_tensor("outb", [R, D_OUT], dt.bfloat16, kind="ExternalOutput")

    with tile.TileContext(nc) as tc:
        with (
            tc.tile_pool(name="const", bufs=1) as const_pool,
            tc.tile_pool(name="gath", bufs=1) as gath_pool,
            tc.tile_pool(name="evac", bufs=4) as evac_pool,
            tc.tile_pool(name="psum", bufs=3, space="PSUM") as psum_pool,
        ):
            # token-index tiles (tiny, land first)
            idx_sb = {}
            for g in active:
                it = const_pool.tile([P, slot_counts[g] // 16], dt.int16,
                                     tag=f"idx{g}")
                nc.sync.dma_start(it[:], idxs[g][:])
                idx_sb[g] = it

            # gathers: rows land transposed, [128, K, C] = emb^T K-tiles
            gath_sb = {}
            for g in active:
                t, base, size = GROUPS[g]
                K = D_PAD[t] // P
                C = slot_counts[g]
                gt = gath_pool.tile([P, K, C], dt.bfloat16, tag=f"g{g}")
                nc.gpsimd.dma_gather(
                    gt[:],
                    embs[t][base:base + size, :],
                    idx_sb[g][:],
                    C,
                    C,
                    D_PAD[t],
                    transpose=True,
                )
                gath_sb[g] = gt

            # resident projections: [Dp, 1024] -> [128, K, 1024]
            proj_sb = []
            for t in range(4):
                K = D_PAD[t] // P
                pt = const_pool.tile([P, K, D_OUT], dt.bfloat16, tag=f"proj{t}")
                nc.sync.dma_start(
                    pt[:], projs[t][:, :].rearrange("(k p) n -> p k n", p=P)
                )
                proj_sb.append(pt)

            # per 128-token chunk: accumulate over K, evacuate, store
            row0 = 0
            n_chunk = 0
            for g in active:
                t, _, _ = GROUPS[g]
                K = D_PAD[t] // P
                C = slot_counts[g]
                for c in range(C // P):
                    rows = min(P, out_counts[g] - c * P)
                    if rows <= 0:
                        continue
                    ps = psum_pool.tile([P, D_OUT], dt.float32, tag="ps")
                    for n in range(2):
                        for kt in range(K):
                            nc.tensor.matmul(
                                ps[:, n * 512:(n + 1) * 512],
                                gath_sb[g][:, kt, c * P:(c + 1) * P],
                                proj_sb[t][:, kt, n * 512:(n + 1) * 512],
                                start=(kt == 0),
                                stop=(kt == K - 1),
                            )
                    ev = evac_pool.tile([P, D_OUT], dt.bfloat16, tag="ev")
                    if n_chunk % 2 == 0:
                        nc.vector.tensor_copy(ev[:], ps[:])
                    else:
                        nc.scalar.copy(ev[:], ps[:])
                    n_chunk += 1
                    nc.sync.dma_start(
                        outb[row0 + c * P: row0 + c * P + rows, :],
                        ev[:rows, :],
                    )
                row0 += out_counts[g]

    nc.finalize()
    return nc


def _prep_tables(emb0, emb1, emb2, emb3, proj0, proj1, proj2, proj3):
    tables = []
    for t, emb in enumerate((emb0, emb1, emb2, emb3)):
        if D_PAD[t] == emb.shape[1]:
            tables.append(np.ascontiguousarray(emb.astype(BF16)))
        else:
            tb = np.zeros((emb.shape[0], D_PAD[t]), BF16)
            tb[:, :emb.shape[1]] = emb.astype(BF16)
            tables.append(tb)
    projTs = []
    for t, proj in enumerate((proj0, proj1, proj2, proj3)):
        pt = np.zeros((D_PAD[t], D_OUT), np.float32)
        pt[:proj.shape[1], :] = (proj.astype(np.float32) * EMB_SCALE).T
        projTs.append(pt.astype(BF16))
    return tables, projTs


def _host_prep(inp):
    """Bucket tokens into groups; compute per-core row/slot counts."""
    flat = np.asarray(inp).reshape(-1).astype(np.int64)

    tbl = np.searchsorted(np.asarray(CUTS[1:]), flat, side="right")
    local = flat - np.asarray(CUTS)[tbl]

    positions = {}
    lidx = {}
    for g, (t, base, size) in enumerate(GROUPS):
        mask = (tbl == t) & (local >= base) & (local < base + size)
        pos = np.nonzero(mask)[0]
        if pos.size:
            positions[g] = pos
            lidx[g] = (local[pos] - base).astype(np.int16)

    active = tuple(sorted(positions.keys()))
    out_counts = {}
    slot_counts = {}
    for g in active:
        n = len(positions[g])
        cg = -(-n // N_CORES)           # ceil(n / 8): rows per core
        out_counts[g] = cg
        slot_counts[g] = max(P, -(-cg // P) * P)
    return flat, active, positions, lidx, out_counts, slot_counts


def _idx_tile(li, slots):
    """int16 [128, slots/16] tile: slot j at [j%16, j//16]; pads read row 0.

    HW's dma_gather (queue 0) reads the indices from partitions 16-31 while
    CoreSim reads 0-15 — write both ranges so either consumer sees them.
    """
    arr = np.zeros((P, slots // 16), np.int16)
    j = np.arange(len(li))
    arr[j % 16, j // 16] = li
    arr[16 + j % 16, j // 16] = li
    return arr


def kernel(inp, emb0, emb1, emb2, emb3, proj0, proj1, proj2, proj3):
    global LAST_RESULTS
    from concourse.bass_utils import run_bass_kernel_spmd

    flat, active, positions, lidx, out_counts, slot_counts = _host_prep(inp)
    T = flat.shape[0]

    tables, projTs = _prep_tables(emb0, emb1, emb2, emb3,
                                  proj0, proj1, proj2, proj3)
    tbl_rows = tuple(tb.shape[0] for tb in tables)

    key = (active, tuple(slot_counts[g] for g in active),
           tuple(out_counts[g] for g in active), tbl_rows)
    nc = _PROGRAM_CACHE.get(key)
    if nc is None:
        nc = _build_program(active, slot_counts, out_counts, tbl_rows)
        _PROGRAM_CACHE[key] = nc

    # per-core index tiles: token slot j of a group reads idx[j%16, j//16]
    in_maps = []
    for k in range(N_CORES):
        m = {}
        for t in range(4):
            m[f"embt{t}"] = tables[t]
            m[f"projt{t}"] = projTs[t]
        for g in active:
            m[f"idx{g}"] = _idx_tile(lidx[g][k::N_CORES], slot_counts[g])
        in_maps.append(m)

    trace = bool(os.environ.get("KERNEL_TRACE"))
    res = run_bass_kernel_spmd(nc, in_maps, core_ids=list(range(N_CORES)),
                               trace=trace)
    LAST_RESULTS = res

    out = np.empty((T, D_OUT), np.float32)
    bases = {}
    r0 = 0
    for g in active:
        bases[g] = r0
        r0 += out_counts[g]
    for k in range(N_CORES):
        ob = np.asarray(res.results[k]["outb"])
        for g in active:
            pos = positions[g][k::N_CORES]
            if pos.size:
                out[pos] = ob[bases[g]:bases[g] + len(pos)].astype(np.float32)

    return out.reshape(*np.asarray(inp).shape, D_OUT)


# revision 6
# speedup vs baseline: 1.2584x; 1.2584x over previous
"""Adaptive embedding (4-bucket) lookup + projection on 8 TRN2 NeuronCores.

Strategy: pure data-parallel over the 16384 tokens (no collectives).
  Host: bucket every token by its embedding table, deduplicate each table to
        the rows actually referenced (<= n_tokens distinct rows, so gather
        indices always fit int16), sort each bucket's tokens by row for HBM
        locality, and deal them evenly across the 8 cores so every core runs
        an identical-shape program.  Tables are pre-cast to bf16 with rows
        padded to a multiple of 128 elements; projections are pre-transposed,
        pre-scaled by sqrt(D) and zero-padded to match.
  Core: one dma_gather(transpose=True) per table pulls that bucket's
        embedding rows from HBM directly into d-on-partitions (matmul lhsT)
        layout; accumulating matmuls against the resident projT produce
        [128 tokens, 1024] in PSUM; DVE/ACT alternate evacuating to bf16 in
        SBUF; plain DMA stores the rows.
  Host: rows are scattered back to original token order and upcast to f32.
"""

import os
import sys

import numpy as np

for _p in ("/opt/trn_rl_repo",):
    if _p not in sys.path:
        sys.path.insert(0, _p)

import ml_dtypes

BF16 = ml_dtypes.bfloat16

N_TOKEN = 267735
CUTS = (0, 20000, 40000, 200000, N_TOKEN)
D_TBL = (1024, 256, 64, 16)
D_PAD = (1024, 256, 128, 128)
D_OUT = 1024
EMB_SCALE = float(D_OUT) ** 0.5
N_CORES = 8
P = 128

_PROGRAM_CACHE = {}
LAST_RESULTS = None  # BassKernelResults of the most recent run (for profiling)


def _build_program(active, slot_counts, out_counts, tbl_rows):
    """Build + compile the per-core Bass program.

    active: tuple of table ids with nonzero token count
    slot_counts / out_counts: per active table — gather slots (mult of 128)
        and output row count (identical on every core)
    tbl_rows: rows of each deduplicated bf16 table
    """
    import concourse.bacc as bacc
    import concourse.mybir as mybir
    import concourse.tile as tile

    dt = mybir.dt
    nc = bacc.Bacc("TRN2", target_bir_lowering=False, debug=False)

    embs = {
        t: nc.dram_tensor(f"embt{t}", [tbl_rows[t], D_PAD[t]], dt.bfloat16,
                          kind="ExternalInput")
        for t in active
    }
    projs = {
        t: nc.dram_tensor(f"projt{t}", [D_PAD[t], D_OUT], dt.bfloat16,
                          kind="ExternalInput")
        for t in active
    }
    total_slots = sum(slot_counts[t] for t in active)
    idx = nc.dram_tensor("idx", [P, total_slots // 16], dt.int16,
                         kind="ExternalInput")
    R = sum(out_counts[t] for t in active)
    outb = nc.dram_tensor("outb", [R, D_OUT], dt.bfloat16, kind="ExternalOutput")

    with tile.TileContext(nc) as tc:
        with (
            tc.tile_pool(name="const", bufs=1) as const_pool,
            tc.tile_pool(name="gath", bufs=1) as gath_pool,
            tc.tile_pool(name="evac", bufs=4) as evac_pool,
            tc.tile_pool(name="psum", bufs=3, space="PSUM") as psum_pool,
        ):
            # all token-index tiles in one small DMA, first in the queue
            idx_sb = const_pool.tile([P, total_slots // 16], dt.int16, tag="idx")
            nc.sync.dma_start(idx_sb[:], idx[:])

            # gathers: rows land transposed, [128, K, C] = emb^T K-tiles.
            # The Q7 gather kernel's index scratch caps num_idxs (~1K crashes
            # on HW) — split big gathers into <=MAX_GATHER column slices.
            MAX_GATHER = 768
            gath_sb = {}
            off = 0
            for t in active:
                K = D_PAD[t] // P
                C = slot_counts[t]
                gt = gath_pool.tile([P, K, C], dt.bfloat16, tag=f"g{t}")
                n_piece = -(-C // MAX_GATHER)
                piece = -(-(C // P) // n_piece) * P
                assert n_piece == 1 or K == 1
                for c0 in range(0, C, piece):
                    cs = min(piece, C - c0)
                    nc.gpsimd.dma_gather(
                        gt[:, :, c0:c0 + cs] if n_piece > 1 else gt[:],
                        embs[t][:, :],
                        idx_sb[:, (off + c0) // 16:(off + c0 + cs) // 16],
                        cs,
                        cs,
                        D_PAD[t],
                        transpose=True,
                    )
                gath_sb[t] = gt
                off += C

            # resident projections: [Dp, 1024] -> [128, K, 1024].
            # Split each into per-K-tile DMAs so the first matmuls only wait
            # for the K-tiles they read.
            proj_sb = {}
            for t in active:
                K = D_PAD[t] // P
                pt = const_pool.tile([P, K, D_OUT], dt.bfloat16, tag=f"proj{t}")
                src = projs[t][:, :].rearrange("(k p) n -> p k n", p=P)
                for k in range(K):
                    nc.sync.dma_start(pt[:, k, :], src[:, k, :])
                proj_sb[t] = pt

            # per 128-token chunk: accumulate over K, evacuate, store
            row0 = 0
            n_chunk = 0
            for t in active:
                K = D_PAD[t] // P
                C = slot_counts[t]
                for c in range(C // P):
                    rows = min(P, out_counts[t] - c * P)
                    if rows <= 0:
                        continue
                    ps = psum_pool.tile([P, D_OUT], dt.float32, tag="ps")
                    for n in range(2):
                        for kt in range(K):
                            nc.tensor.matmul(
                                ps[:, n * 512:(n + 1) * 512],
                                gath_sb[t][:, kt, c * P:(c + 1) * P],
                                proj_sb[t][:, kt, n * 512:(n + 1) * 512],
                                start=(kt == 0),
                                stop=(kt == K - 1),
                            )
                    ev = evac_pool.tile([P, D_OUT], dt.bfloat16, tag="ev")
                    if n_chunk % 2 == 0:
                        nc.vector.tensor_copy(ev[:], ps[:])
                    else:
                        nc.scalar.copy(ev[:], ps[:])
                    n_chunk += 1
                    nc.sync.dma_start(
                        outb[row0 + c * P: row0 + c * P + rows, :],
                        ev[:rows, :],
                    )
                row0 += out_counts[t]

    nc.finalize()
    return nc


def _host_prep(inp):
    """Bucket tokens by table; dedup rows; sort by row; per-core counts."""
    flat = np.asarray(inp).reshape(-1).astype(np.int64)

    tbl = np.searchsorted(np.asarray(CUTS[1:]), flat, side="right")
    local = flat - np.asarray(CUTS)[tbl]

    positions = {}
    lidx = {}
    uniq = {}
    for t in range(4):
        pos = np.nonzero(tbl == t)[0]
        if not pos.size:
            continue
        rows = local[pos]
        u, inv = np.unique(rows, return_inverse=True)
        order = np.argsort(inv, kind="stable")   # sort tokens by table row
        positions[t] = pos[order]
        lidx[t] = inv[order].astype(np.int16)
        uniq[t] = u

    active = tuple(sorted(positions.keys()))
    out_counts = {}
    slot_counts = {}
    for t in active:
        n = len(positions[t])
        cg = -(-n // N_CORES)           # ceil(n / 8): rows per core
        out_counts[t] = cg
        slot_counts[t] = max(P, -(-cg // P) * P)
    return flat, active, positions, lidx, uniq, out_counts, slot_counts


def _idx_tensor(active, lidx, slot_counts, core):
    """Combined int16 [128, total_slots/16] tile for one core.

    Slot j of a group at [j%16, j//16] within the group's column window;
    pads read row 0.  HW's dma_gather (queue 0) reads the indices from
    partitions 16-31 while CoreSim reads 0-15 — write both ranges.
    """
    total = sum(slot_counts[t] for t in active)
    arr = np.zeros((P, total // 16), np.int16)
    off = 0
    for t in active:
        li = lidx[t][core::N_CORES]
        j = np.arange(len(li))
        arr[j % 16, off // 16 + j // 16] = li
        arr[16 + j % 16, off // 16 + j // 16] = li
        off += slot_counts[t]
    return arr


def _prep_compact_tables(active, uniq, raw_tables, raw_projs):
    tables = {}
    projTs = {}
    for t in active:
        emb = raw_tables[t]
        sel = np.asarray(emb, dtype=np.float32)[uniq[t]]
        tb = np.zeros((len(uniq[t]), D_PAD[t]), BF16)
        tb[:, :emb.shape[1]] = sel.astype(BF16)
        tables[t] = tb
        proj = raw_projs[t]
        pt = np.zeros((D_PAD[t], D_OUT), np.float32)
        pt[:proj.shape[1], :] = (np.asarray(proj, np.float32) * EMB_SCALE).T
        projTs[t] = pt.astype(BF16)
    return tables, projTs


def kernel(inp, emb0, emb1, emb2, emb3, proj0, proj1, proj2, proj3):
    global LAST_RESULTS
    from concourse.bass_utils import run_bass_kernel_spmd

    flat, active, positions, lidx, uniq, out_counts, slot_counts = \
        _host_prep(inp)
    T = flat.shape[0]

    tables, projTs = _prep_compact_tables(
        active, uniq, (emb0, emb1, emb2, emb3), (proj0, proj1, proj2, proj3))
    tbl_rows = {t: tables[t].shape[0] for t in active}

    key = (active, tuple(slot_counts[t] for t in active),
           tuple(out_counts[t] for t in active),
           tuple(tbl_rows[t] for t in active))
    nc = _PROGRAM_CACHE.get(key)
    if nc is None:
        nc = _build_program(active, slot_counts, out_counts, tbl_rows)
        _PROGRAM_CACHE[key] = nc

    in_maps = []
    for k in range(N_CORES):
        m = {}
        for t in active:
            m[f"embt{t}"] = tables[t]
            m[f"projt{t}"] = projTs[t]
        m["idx"] = _idx_tensor(active, lidx, slot_counts, k)
        in_maps.append(m)

    trace = bool(os.environ.get("KERNEL_TRACE"))
    res = run_bass_kernel_spmd(nc, in_maps, core_ids=list(range(N_CORES)),
                               trace=trace)
    LAST_RESULTS = res

    out = np.empty((T, D_OUT), np.float32)
    bases = {}
    r0 = 0
    for t in active:
        bases[t] = r0
        r0 += out_counts[t]
    for k in range(N_CORES):
        ob = np.asarray(res.results[k]["outb"])
        for t in active:
            pos = positions[t][k::N_CORES]
            if pos.size:
                out[pos] = ob[bases[t]:bases[t] + len(pos)].astype(np.float32)

    return out.reshape(*np.asarray(inp).shape, D_OUT)
